# revision 29
# baseline (speedup 1.0000x reference)
"""Trainium2 Bass kernel for the Basicgate multivoxel attention module.

The voxel-features -> attention-logit chain is linear and collapses to

  logit(r,w) = sum_{s,k} T_s[k, r+dy-1, w+dx-1]          (point taps)
             + sum_k S[k] * gated(r+dy-1, w+dx-1)        (gated 3x3)
             + edge constants;     out = img * sigmoid(logit + sp_b)

with per point p of set s at cell (h,w): T_s[:, h, w] = B_s @ x_p,
B_s = V@W2@W0 / V@W2@W1 / V@W2, x_p = concat(feat, coord), V the 3x3 taps,
gated = w3.img + b3, S[k] = sum_c V[k,c].

Sharding: H split across 8 cores (32 owned rows + 1 halo row per side ->
34 local rows).  Points bucketed on host by (core, set, local row).  No
collectives; host gathers the output.

Device pipeline per core (all dense data bf16, accumulation f32 in PSUM):
  A. img slab (256,34,704) f32 is DMA'd ONCE via the gpsimd (SWDGE) queue
     with an f32->bf16 cast into a persistent SBUF slab (95KB/partition).
  B. token matmuls: host lays x out so each PE matmul's stationary is one
     token slot across all 34 rows ([K,34]); moving is B_s^T [K,9].  PSUM
     output [34 rows, 9 taps] is DVE-copied to bf16 staging and compacted
     into per-tap token streams.  Engine SBUF accesses must start at
     partition 0/32/64/96, so set 0 lives at base 0 and set 1 at base 64
     of the 112-partition scatter tiles (set 1 shifted there by a small
     SBUF->SBUF DMA -- DMAs are exempt); set 2 gets its own 48-partition
     tiles at base 0.
  C. gpsimd.local_scatter x18 (tap x tile-group): dst T3_k[row, 706]
     gets data_k placed at host-provided column indices.  This Q7 kernel
     vectorizes the scatter in GPSIMD local RAM (~15 cycles per 32
     tokens, one DSP per 16 partitions), zero-fills the dst, and skips
     negative (padding) indices -- it replaces the baseline's
     dma_scatter_add (7ns/token serial desc-gen) + 18MB DRAM zeroing +
     36B-descriptor readback, which dominated the baseline runtime.
  D. gated map: w3 . img streamed from the bf16 slab on PE.
  E. logit: 27 T taps + 9 gated taps accumulate into PSUM as matmuls with
     row-shift selection stationaries (em_dy, S_k-scaled for gated).
     Edge-constant column/row fixes, then sigmoid (ACT, bias=rowfix) ->
     att bf16.
  F. att broadcast to 128 partitions (ones-stationary matmul), DVE
     multiply with the bf16 img slab, f32 out stores split across the two
     HWDGE queues.
"""

import numpy as np

# ---- problem constants (hardcoded per contract) ----
C_IMG = 256
H, W = 256, 704
CH = [32, 64, 128]
COUT = 131
N_CORES = 8
R = 32            # owned rows per core
L = 34            # local rows incl 1-row halo each side
WP = W + 2        # padded width (token cols live in [1, 705))
KS = [CH[0] + 3, CH[1] + 3, COUT]   # 35, 67, 131
P102 = 3 * L      # (set, row) partitions
P112 = 112        # padded to a multiple of 16 for local_scatter channels
TPT = 28          # token slots per PSUM tile (28*9 = 252 f32 cols)

LAST_RESULT = None  # stash of BassKernelResults for the test harness


def _fold_weights(inputs):
    f8 = np.float64
    W0 = inputs["rd0_w"][:, :, 0, 0].astype(f8)   # (131, 35)
    W1 = inputs["rd1_w"][:, :, 0, 0].astype(f8)   # (131, 67)
    W2 = inputs["rd2_w"][:, :, 0, 0].astype(f8)   # (131, 131)
    w3 = inputs["rd3_w"][0, :, 0, 0].astype(f8)   # (256,)
    b0 = inputs["rd0_b"].astype(f8)
    b1 = inputs["rd1_b"].astype(f8)
    b2 = inputs["rd2_b"].astype(f8)
    b3 = float(inputs["rd3_b"][0])
    spb = float(inputs["sp_b"][0])
    # V[k=dy*3+dx, c] = sp_w[0, c, dy, dx]
    V = inputs["sp_w"][0].astype(f8).transpose(1, 2, 0).reshape(9, COUT)
    B = [V @ (W2 @ W0), V @ (W2 @ W1), V @ W2]
    cc = V @ (W2 @ (b0 + b1) + b2)   # (9,)
    S = V.sum(axis=1)                # (9,)
    return dict(B=B, cc=cc, S=S, C_all=float(cc.sum()),
                w3=w3, b3=b3, spb=spb)


def _build_program(Ms, M, b3):
    """Ms = per-set token-slot capacity per row (identical across cores),
    M = max(Ms[0], Ms[1]) and M2 = Ms[2], both even (local_scatter
    num_idxs)."""
    import concourse.bacc as bacc
    import concourse.mybir as mybir
    import concourse.tile as tile

    f32 = mybir.dt.float32
    bf16 = mybir.dt.bfloat16
    i16 = mybir.dt.int16
    Alu = mybir.AluOpType
    Act = mybir.ActivationFunctionType

    import os
    no_cast = os.environ.get("BG_NO_CAST", "0") == "1"
    no_lscat = os.environ.get("BG_NO_LSCAT", "0") == "1"
    safe_dma = os.environ.get("BG_SAFE_DMA", "0") == "1"

    M2 = Ms[2]
    SEGS = ((0, 512), (512, 192))
    ESEGS = ((0, 512), (512, 512), (1024, 384))
    # (tile-group, partition base) per set: engine SBUF reads must start
    # at partition 0/32/64/96, so sets live at A:0, A:64, B:0.
    SETP = [(0, 0), (0, 64), (1, 0)]

    nc = bacc.Bacc("TRN2", target_bir_lowering=False, debug=False,
                   num_devices=N_CORES)

    img = nc.dram_tensor("img", [C_IMG, L, W], f32, kind="ExternalInput").ap()
    xs = [nc.dram_tensor(f"x{i}", [min(KS[i], 128), Ms[i] * L], bf16,
                         kind="ExternalInput").ap() for i in range(3)]
    x2b = nc.dram_tensor("x2b", [KS[2] - 128, Ms[2] * L], bf16,
                         kind="ExternalInput").ap()
    idxsAd = nc.dram_tensor("idxsA", [P112, M], i16, kind="ExternalInput").ap()
    idxsBd = nc.dram_tensor("idxsB", [48, M2], i16, kind="ExternalInput").ap()
    bts = [nc.dram_tensor(f"bt{i}", [min(KS[i], 128), 9], bf16,
                          kind="ExternalInput").ap() for i in range(3)]
    bt2b = nc.dram_tensor("bt2b", [KS[2] - 128, 9], bf16,
                          kind="ExternalInput").ap()
    w3d = nc.dram_tensor("w3", [C_IMG // 2, 2], bf16, kind="ExternalInput").ap()
    emsd = nc.dram_tensor("ems", [L, 384], bf16, kind="ExternalInput").ap()
    rowmaskd = nc.dram_tensor("rowmask", [L, 1], f32, kind="ExternalInput").ap()
    rowfixd = nc.dram_tensor("rowfix", [R, 1], f32, kind="ExternalInput").ap()
    colfix0d = nc.dram_tensor("colfix0", [R, 1], f32, kind="ExternalInput").ap()
    colfix1d = nc.dram_tensor("colfix1", [R, 1], f32, kind="ExternalInput").ap()
    out = nc.dram_tensor("out", [C_IMG, R, W], f32, kind="ExternalOutput").ap()

    with tile.TileContext(nc) as tc:
        with (
            tc.tile_pool(name="persist", bufs=1) as pp,
            tc.tile_pool(name="work", bufs=2) as wp,
            tc.tile_pool(name="io", bufs=2) as ip,
            tc.tile_pool(name="pstok", bufs=2, space="PSUM") as pst,
            tc.tile_pool(name="pslg", bufs=1, space="PSUM") as psl,
            tc.tile_pool(name="psatt", bufs=2, space="PSUM") as psa,
        ):
            # ---- persistent tiles ----
            imgsb = pp.tile([128, 2 * L * W], bf16, tag="imgsb")
            img3 = imgsb[:].rearrange("p (hh l w) -> p hh l w", hh=2, l=L)
            T3a = [pp.tile([P112, WP], bf16, tag=f"T3a_{k}", name=f"T3a_{k}")
                   for k in range(9)]
            T3b = [pp.tile([48, WP], bf16, tag=f"T3b_{k}", name=f"T3b_{k}")
                   for k in range(9)]
            dataA = [pp.tile([P112, M], bf16, tag=f"dA_{k}", name=f"dA_{k}")
                     for k in range(9)]
            dataB = [pp.tile([48, M2], bf16, tag=f"dB_{k}", name=f"dB_{k}")
                     for k in range(9)]
            idxsA_t = pp.tile([P112, M], i16, tag="idxsA")
            idxsB_t = pp.tile([48, M2], i16, tag="idxsB")
            bt_t = []
            for s in range(3):
                t = pp.tile([min(KS[s], 128), 9], bf16, tag=f"bt{s}",
                            name=f"bt{s}")
                nc.scalar.dma_start(out=t[:], in_=bts[s][:])
                bt_t.append(t)
            bt2b_t = pp.tile([KS[2] - 128, 9], bf16, tag="bt2b")
            nc.scalar.dma_start(out=bt2b_t[:], in_=bt2b[:])
            w3_t = pp.tile([C_IMG // 2, 2], bf16, tag="w3")
            nc.scalar.dma_start(out=w3_t[:], in_=w3d[:])
            # em stationaries live at partition base 0 AND a copy at base
            # 64: HW requires fmap and weight to start at the same SBUF
            # partition, and set 1's T3/data live at base 64.
            ems = pp.tile([98, 384], bf16, tag="ems")
            nc.scalar.dma_start(out=ems[0:L, :], in_=emsd[:])
            nc.scalar.dma_start(out=ems[64:64 + L, :], in_=emsd[:])
            rowmask_t = pp.tile([L, 1], f32, tag="rowmask")
            nc.scalar.dma_start(out=rowmask_t[:], in_=rowmaskd[:])
            rowfix_t = pp.tile([R, 1], f32, tag="rowfix")
            nc.scalar.dma_start(out=rowfix_t[:], in_=rowfixd[:])
            colfix0_t = pp.tile([R, 1], f32, tag="colfix0")
            nc.scalar.dma_start(out=colfix0_t[:], in_=colfix0d[:])
            colfix1_t = pp.tile([R, 1], f32, tag="colfix1")
            nc.scalar.dma_start(out=colfix1_t[:], in_=colfix1d[:])
            nc.sync.dma_start(out=idxsA_t[:], in_=idxsAd[:])
            nc.sync.dma_start(out=idxsB_t[:], in_=idxsBd[:])

            ones_t = pp.tile([1, 128], bf16, tag="ones")
            nc.vector.memset(ones_t[:], 1.0)
            # local_scatter streams the full data tiles; zero the pad regions
            for k in range(9):
                nc.vector.memset(dataA[k][:], 0.0)
                nc.vector.memset(dataB[k][:], 0.0)
            gmap = pp.tile([L, WP], bf16, tag="gmap")
            nc.vector.memset(gmap[:], 0.0)
            att = pp.tile([R, W], bf16, tag="att")

            # ---- A: img slab cast-load on the SWDGE queue ----
            if no_cast:
                for hh in range(2):
                    for l in range(L):
                        imgf = ip.tile([128, W], f32, tag="out")
                        nc.sync.dma_start(
                            out=imgf[:],
                            in_=img[hh * 128:(hh + 1) * 128, l, :])
                        nc.scalar.copy(out=img3[:, hh, l, :], in_=imgf[:])
            else:
                row_chunks = [(0, 9), (9, 18), (18, 26), (26, L)]
                for r0, r1 in row_chunks:
                    for hh in range(2):
                        nc.gpsimd.dma_start(
                            out=img3[:, hh, r0:r1, :],
                            in_=img[hh * 128:(hh + 1) * 128, r0:r1, :
                                    ].rearrange("c l w -> c (l w)"))

            # ---- B: token matmuls -> per-set staging -> per-tap data ----
            for s in range(3):
                grp, pb = SETP[s]
                NTs = -(-Ms[s] // TPT)
                stg = wp.tile([L, M * 9], bf16, tag="stg")
                for j in range(NTs):
                    tok = pst.tile([L, TPT * 9], f32, tag="tok")
                    t0 = TPT * j
                    t1 = min(t0 + TPT, Ms[s])
                    cols = (t1 - t0) * L
                    ka = min(KS[s], 128)
                    xt = wp.tile([128, TPT * L], bf16, tag=f"x{s}")
                    nc.sync.dma_start(out=xt[:ka, :cols],
                                      in_=xs[s][:, t0 * L:t1 * L])
                    xtb = None
                    if KS[s] > 128:
                        xtb = wp.tile([KS[s] - 128, TPT * L], bf16, tag="x2b")
                        nc.sync.dma_start(out=xtb[:, :cols],
                                          in_=x2b[:, t0 * L:t1 * L])
                    for t in range(t0, t1):
                        c0 = (t - t0) * L
                        po = (t - t0) * 9
                        dst = tok[:, po:po + 9]
                        if xtb is None:
                            nc.tensor.matmul(dst, xt[:ka, c0:c0 + L],
                                             bt_t[s][:], start=True, stop=True)
                        else:
                            nc.tensor.matmul(dst, xt[:, c0:c0 + L],
                                             bt_t[s][:], start=True, stop=False)
                            nc.tensor.matmul(dst, xtb[:, c0:c0 + L],
                                             bt2b_t[:], start=False, stop=True)
                    nc.vector.tensor_copy(
                        out=stg[:, j * TPT * 9:j * TPT * 9 + (t1 - t0) * 9],
                        in_=tok[:, :(t1 - t0) * 9])
                stg3 = stg[:].rearrange("p (t e) -> p t e", e=9)
                for k in range(9):
                    dtile = dataA[k] if grp == 0 else dataB[k]
                    if pb == 0:
                        nc.vector.tensor_copy(out=dtile[0:L, :Ms[s]],
                                              in_=stg3[:, :Ms[s], k])
                    else:
                        # engine writes can't start at partition 64+...;
                        # compact at base 0, then partition-shift by DMA
                        dsk = wp.tile([L, M], bf16, tag="dsk")
                        nc.vector.tensor_copy(out=dsk[:, :Ms[s]],
                                              in_=stg3[:, :Ms[s], k])
                        nc.scalar.dma_start(out=dtile[pb:pb + L, :Ms[s]],
                                            in_=dsk[:, :Ms[s]])

            # ---- C: per-tap local scatter (Pool/Q7) ----
            for k in range(9):
                if no_lscat:
                    nc.vector.memset(T3a[k][:], 0.0)
                    nc.vector.memset(T3b[k][:], 0.0)
                    continue
                nc.gpsimd.local_scatter(
                    out_ap=T3a[k][:, :WP],
                    data_ap=dataA[k][:, :M],
                    idxs_ap=idxsA_t[:, :M],
                    channels=P112,
                    num_elems=WP,
                    num_idxs=M,
                )
                nc.gpsimd.local_scatter(
                    out_ap=T3b[k][:, :WP],
                    data_ap=dataB[k][:, :M2],
                    idxs_ap=idxsB_t[:, :M2],
                    channels=48,
                    num_elems=WP,
                    num_idxs=M2,
                )

            # ---- D: gated map ----
            for rc in range(L // 2):
                gstage = wp.tile([1, 2 * W], bf16, tag="gstage")
                for off, n in ESEGS:
                    gp = pst.tile([1, 512], f32, tag="g")
                    for hh in range(2):
                        nc.tensor.matmul(
                            gp[:, :n],
                            w3_t[:, hh:hh + 1],
                            img3[:, hh, 2 * rc:2 * rc + 2, :].rearrange(
                                "p l w -> p (l w)")[:, off:off + n],
                            start=(hh == 0), stop=(hh == 1))
                    nc.scalar.copy(out=gstage[:, off:off + n], in_=gp[:, :n])
                if safe_dma:
                    for r01 in range(2):
                        nc.scalar.dma_start(
                            out=gmap[2 * rc + r01:2 * rc + r01 + 1, 1:1 + W],
                            in_=gstage[:, r01 * W:(r01 + 1) * W])
                else:
                    nc.scalar.dma_start(
                        out=gmap[2 * rc:2 * rc + 2, 1:1 + W],
                        in_=gstage[:])
            # gmap = (gmap + b3) * rowmask on interior columns
            nc.vector.tensor_scalar(
                out=gmap[:, 1:1 + W], in0=gmap[:, 1:1 + W],
                scalar1=float(b3), scalar2=rowmask_t[:, 0:1],
                op0=Alu.add, op1=Alu.mult)

            # ---- E: logit assembly (em row-shift stationaries) ----
            lg = psl.tile([R, W], f32, tag="lg")
            nmm = [0, 0]
            total = 27 + 9
            for gi, (off, n) in enumerate(SEGS):
                for k in range(9):
                    dy, dx = divmod(k, 3)
                    for s in range(3):
                        grp, pb = SETP[s]
                        src = T3a[k] if grp == 0 else T3b[k]
                        nmm[gi] += 1
                        nc.tensor.matmul(
                            lg[:, off:off + n],
                            ems[pb:pb + L, dy * R:dy * R + R],
                            src[pb:pb + L, dx + off:dx + off + n],
                            start=(nmm[gi] == 1), stop=False)
                    nmm[gi] += 1
                    nc.tensor.matmul(
                        lg[:, off:off + n],
                        ems[0:L, 96 + k * R:96 + k * R + R],
                        gmap[:, dx + off:dx + off + n],
                        start=False, stop=(nmm[gi] == total))
            nc.vector.tensor_tensor(out=lg[:, 0:1], in0=lg[:, 0:1],
                                    in1=colfix0_t[:, 0:1], op=Alu.add)
            nc.vector.tensor_tensor(out=lg[:, W - 1:W], in0=lg[:, W - 1:W],
                                    in1=colfix1_t[:, 0:1], op=Alu.add)
            # rowfix carries C_all + sp_b + row-edge constants
            for off, n in SEGS:
                nc.scalar.activation(att[:, off:off + n], lg[:, off:off + n],
                                     Act.Sigmoid, bias=rowfix_t[:, 0:1],
                                     scale=1.0)

            # ---- F: broadcast multiply + store ----
            for rc in range(R // 2):
                # flatten 2 att rows into one partition (sizes match; the
                # DMA walks both APs in row-major order)
                a1p = wp.tile([1, 2 * W], bf16, tag="a1p")
                if safe_dma:
                    for r01 in range(2):
                        nc.scalar.dma_start(
                            out=a1p[:, r01 * W:(r01 + 1) * W],
                            in_=att[2 * rc + r01:2 * rc + r01 + 1, :])
                else:
                    nc.scalar.dma_start(out=a1p[:],
                                        in_=att[2 * rc:2 * rc + 2, :])
                attb_sb = wp.tile([128, 2 * W], bf16, tag="attb_sb")
                for off, n in ESEGS:
                    ab = psa.tile([128, 512], f32, tag="attb")
                    nc.tensor.matmul(
                        ab[:, :n],
                        ones_t[:],
                        a1p[:, off:off + n],
                        start=True, stop=True)
                    nc.scalar.copy(out=attb_sb[:, off:off + n], in_=ab[:, :n])
                for hh in range(2):
                    for r01 in range(2):
                        ot = ip.tile([128, W], f32, tag="out")
                        nc.vector.tensor_tensor(
                            out=ot[:],
                            in0=img3[:, hh, 1 + 2 * rc + r01, :],
                            in1=attb_sb[:, r01 * W:(r01 + 1) * W],
                            op=Alu.mult)
                        eng = nc.sync if hh == 0 else nc.scalar
                        eng.dma_start(
                            out=out[hh * 128:(hh + 1) * 128,
                                    2 * rc + r01, :],
                            in_=ot[:])

    nc.compile()
    return nc


def _prepare(inputs):
    """Host-side fold + shard. Returns (Ms, M, b3, in_maps)."""
    import ml_dtypes
    bf = ml_dtypes.bfloat16

    fold = _fold_weights(inputs)
    cc, S = fold["cc"], fold["S"]

    grids = [np.asarray(inputs[f"img_grid_{i}"]) for i in range(3)]
    feats = [np.asarray(inputs[f"voxel_feat_{i}"]) for i in range(3)]
    coords = [np.asarray(inputs[f"voxel_coord_{i}"]) for i in range(3)]
    img_feat = np.asarray(inputs["img_feat"])

    # bucket tokens by (core, set, local row)
    sels = []          # sels[c][s] -> token ids
    hls = []           # matching local rows
    for c in range(N_CORES):
        lo = R * c - 1
        per, perh = [], []
        for s in range(3):
            rows = grids[s][:, 1]
            sel = np.nonzero((rows >= lo) & (rows < lo + L))[0]
            per.append(sel)
            perh.append(rows[sel] - lo)
        sels.append(per)
        hls.append(perh)

    Ms = []
    for s in range(3):
        mx = 0
        for c in range(N_CORES):
            if len(hls[c][s]):
                mx = max(mx, int(np.bincount(hls[c][s], minlength=L).max()))
        Ms.append(mx + (mx & 1))
    M = max(Ms[0], Ms[1])
    SETP = [(0, 0), (0, 64), (1, 0)]

    # em row-shift stationaries: [E_dy0|E_dy1|E_dy2 | S_0*E_dy(0) ...]
    ems = np.zeros((L, 384), np.float32)
    for k in range(9):
        dy = k // 3
        for r in range(R):
            ems[r + dy, dy * R + r] = 1.0
            ems[r + dy, 96 + k * R + r] = np.float32(S[k])

    in_maps = []
    for c in range(N_CORES):
        lo = R * c - 1
        m = {}
        slab = np.zeros((C_IMG, L, W), np.float32)
        g0, g1 = max(lo, 0), min(lo + L, H)
        slab[:, g0 - lo:g1 - lo, :] = img_feat[:, g0:g1, :]
        m["img"] = slab

        idxA = np.full((P112, M), -1, np.int16)
        idxB = np.full((48, Ms[2]), -1, np.int16)
        for s in range(3):
            grp, pb = SETP[s]
            sel = sels[c][s]
            hl = hls[c][s].astype(np.int64)
            wl = grids[s][sel, 0].astype(np.int64) + 1
            # slot position of each token within its row
            order = np.argsort(hl, kind="stable")
            hs = hl[order]
            slot = np.arange(len(hs)) - np.searchsorted(hs, hs, side="left")
            x = np.zeros((KS[s], Ms[s] * L), np.float32)
            if len(sel):
                xv = np.concatenate([feats[s][sel], coords[s][sel]],
                                    axis=1).T  # (K, n)
                cols = slot * L + hs
                x[:, cols] = xv[:, order]
                tgt = idxA if grp == 0 else idxB
                tgt[pb + hs, slot] = wl[order].astype(np.int16)
            xb = x.astype(bf)
            if KS[s] > 128:
                m[f"x{s}"] = np.ascontiguousarray(xb[:128])
                m["x2b"] = np.ascontiguousarray(xb[128:])
            else:
                m[f"x{s}"] = xb
        m["idxsA"] = idxA
        m["idxsB"] = idxB
        for s in range(3):
            bt = fold["B"][s].T.astype(np.float32)  # (K, 9)
            if KS[s] > 128:
                m[f"bt{s}"] = np.ascontiguousarray(bt[:128]).astype(bf)
                m["bt2b"] = np.ascontiguousarray(bt[128:]).astype(bf)
            else:
                m[f"bt{s}"] = bt.astype(bf)
        m["w3"] = np.ascontiguousarray(
            fold["w3"].astype(np.float32).reshape(2, 128).T).astype(bf)
        m["ems"] = ems.astype(bf)
        rowmask = np.zeros((L, 1), np.float32)
        rowmask[g0 - lo:g1 - lo] = 1.0
        m["rowmask"] = rowmask
        # rowfix: C_all + sp_b + row-edge constants (used as sigmoid bias)
        rowfix = np.full((R, 1), fold["C_all"] + fold["spb"], np.float64)
        colfix0 = np.full((R, 1), -(cc[0] + cc[3] + cc[6]))
        colfix1 = np.full((R, 1), -(cc[2] + cc[5] + cc[8]))
        for hloc in range(R):
            g = R * c + hloc
            if g == 0:
                rowfix[hloc] += -(cc[0] + cc[1] + cc[2])
                colfix0[hloc] += cc[0]
                colfix1[hloc] += cc[2]
            if g == H - 1:
                rowfix[hloc] += -(cc[6] + cc[7] + cc[8])
                colfix0[hloc] += cc[6]
                colfix1[hloc] += cc[8]
        m["rowfix"] = rowfix.astype(np.float32)
        m["colfix0"] = colfix0.astype(np.float32)
        m["colfix1"] = colfix1.astype(np.float32)
        in_maps.append(m)
    return Ms, M, fold["b3"], in_maps


def kernel(**inputs):
    global LAST_RESULT
    from concourse.bass_utils import run_bass_kernel_spmd

    Ms, M, b3, in_maps = _prepare(inputs)
    nc = _build_program(Ms, M, b3)
    res = run_bass_kernel_spmd(nc, in_maps, core_ids=list(range(N_CORES)))
    LAST_RESULT = res
    out = np.concatenate(
        [res.results[c]["out"] for c in range(N_CORES)], axis=1)
    return np.ascontiguousarray(out.astype(np.float32))


# revision 37
# speedup vs baseline: 3.1902x; 3.1902x over previous
"""Trainium2 Bass kernel for the Basicgate multivoxel attention module.

The voxel-features -> attention-logit chain is linear and collapses to

  logit(r,w) = sum_{s,k} T_s[k, r+dy-1, w+dx-1]          (point taps)
             + sum_k S[k] * gated(r+dy-1, w+dx-1)        (gated 3x3)
             + edge constants;     out = img * sigmoid(logit + sp_b)

with per point p of set s at cell (h,w): T_s[:, h, w] = B_s @ x_p,
B_s = V@W2@W0 / V@W2@W1 / V@W2, x_p = concat(feat, coord), V the 3x3 taps,
gated = w3.img + b3, S[k] = sum_c V[k,c].

Sharding: H split across 8 cores (32 owned rows + 1 halo row per side ->
34 local rows).  Points bucketed on host by (core, set, local row).  No
collectives; host gathers the output.

Device pipeline per core (all dense data bf16, accumulation f32 in PSUM):
  A. img slab (256,34,704) f32 is DMA'd ONCE via the gpsimd (SWDGE) queue
     with an f32->bf16 cast into a persistent SBUF slab (95KB/partition).
  B. token matmuls: host lays x out so each PE matmul's stationary is one
     token slot across all 34 rows ([K,34]); moving is B_s^T [K,9].  PSUM
     output [34 rows, 9 taps] is DVE-copied to bf16 staging and compacted
     into per-tap token streams data_k[row, slot] (all at partition base
     0 -- engine SBUF accesses must start at partition 0/32/64/96, and
     matmul operands at base 64 fault on HW).
  C. gpsimd.local_scatter x27 (tap x set): a [48,706] temp tile gets
     data_k placed at host-provided column indices; DVE accumulates the
     three sets into T3sum_k.  The Q7 scatter kernel vectorizes in GPSIMD
     local RAM (~15 cycles per 32 tokens, one DSP per 16 partitions),
     zero-fills the dst, and skips negative (padding) indices -- it
     replaces the baseline's dma_scatter_add (7ns/token serial desc-gen)
     + 18MB DRAM zeroing + 36B-descriptor readback, which dominated the
     baseline runtime.
  D. gated map: w3 . img streamed from the bf16 slab on PE.
  E. logit: 9 T taps + 9 gated taps accumulate into PSUM as matmuls with
     row-shift selection stationaries (em_dy, S_k-scaled for gated).
     Edge-constant column/row fixes, then sigmoid (ACT, bias=rowfix) ->
     att bf16.
  F. att broadcast to 128 partitions (ones-stationary matmul), DVE
     multiply with the bf16 img slab, f32 out stores split across the two
     HWDGE queues.
"""

import numpy as np

# ---- problem constants (hardcoded per contract) ----
C_IMG = 256
H, W = 256, 704
CH = [32, 64, 128]
COUT = 131
N_CORES = 8
R = 32            # owned rows per core
L = 34            # local rows incl 1-row halo each side
WP = W + 2        # padded width (token cols live in [1, 705))
KS = [CH[0] + 3, CH[1] + 3, COUT]   # 35, 67, 131
P102 = 3 * L      # (set, row) partitions
P112 = 112        # padded to a multiple of 16 for local_scatter channels
TPT = 28          # token slots per PSUM tile (28*9 = 252 f32 cols)

LAST_RESULT = None  # stash of BassKernelResults for the test harness


def _fold_weights(inputs):
    f8 = np.float64
    W0 = inputs["rd0_w"][:, :, 0, 0].astype(f8)   # (131, 35)
    W1 = inputs["rd1_w"][:, :, 0, 0].astype(f8)   # (131, 67)
    W2 = inputs["rd2_w"][:, :, 0, 0].astype(f8)   # (131, 131)
    w3 = inputs["rd3_w"][0, :, 0, 0].astype(f8)   # (256,)
    b0 = inputs["rd0_b"].astype(f8)
    b1 = inputs["rd1_b"].astype(f8)
    b2 = inputs["rd2_b"].astype(f8)
    b3 = float(inputs["rd3_b"][0])
    spb = float(inputs["sp_b"][0])
    # V[k=dy*3+dx, c] = sp_w[0, c, dy, dx]
    V = inputs["sp_w"][0].astype(f8).transpose(1, 2, 0).reshape(9, COUT)
    B = [V @ (W2 @ W0), V @ (W2 @ W1), V @ W2]
    cc = V @ (W2 @ (b0 + b1) + b2)   # (9,)
    S = V.sum(axis=1)                # (9,)
    return dict(B=B, cc=cc, S=S, C_all=float(cc.sum()),
                w3=w3, b3=b3, spb=spb)


def _build_program(Ms, M, b3):
    """Ms = per-set token-slot capacity per row (identical across cores),
    M = max(Ms[0], Ms[1]) and M2 = Ms[2], both even (local_scatter
    num_idxs)."""
    import concourse.bacc as bacc
    import concourse.mybir as mybir
    import concourse.tile as tile

    f32 = mybir.dt.float32
    bf16 = mybir.dt.bfloat16
    i16 = mybir.dt.int16
    Alu = mybir.AluOpType
    Act = mybir.ActivationFunctionType

    import os
    no_cast = os.environ.get("BG_NO_CAST", "0") == "1"
    no_lscat = os.environ.get("BG_NO_LSCAT", "0") == "1"
    safe_dma = os.environ.get("BG_SAFE_DMA", "0") == "1"

    SEGS = ((0, 512), (512, 192))
    ESEGS = ((0, 512), (512, 512), (1024, 384))

    nc = bacc.Bacc("TRN2", target_bir_lowering=False, debug=False,
                   num_devices=N_CORES)

    img = nc.dram_tensor("img", [C_IMG, L, W], f32, kind="ExternalInput").ap()
    xs = [nc.dram_tensor(f"x{i}", [min(KS[i], 128), Ms[i] * L], bf16,
                         kind="ExternalInput").ap() for i in range(3)]
    x2b = nc.dram_tensor("x2b", [KS[2] - 128, Ms[2] * L], bf16,
                         kind="ExternalInput").ap()
    idxs_d = [nc.dram_tensor(f"idxs{s}", [48, Ms[s]], i16,
                             kind="ExternalInput").ap() for s in range(3)]
    bts = [nc.dram_tensor(f"bt{i}", [min(KS[i], 128), 9], bf16,
                          kind="ExternalInput").ap() for i in range(3)]
    bt2b = nc.dram_tensor("bt2b", [KS[2] - 128, 9], bf16,
                          kind="ExternalInput").ap()
    w3d = nc.dram_tensor("w3", [C_IMG // 2, 2], bf16, kind="ExternalInput").ap()
    emsd = nc.dram_tensor("ems", [L, 384], bf16, kind="ExternalInput").ap()
    rowmaskd = nc.dram_tensor("rowmask", [L, 1], f32, kind="ExternalInput").ap()
    rowfixd = nc.dram_tensor("rowfix", [R, 1], f32, kind="ExternalInput").ap()
    colfix0d = nc.dram_tensor("colfix0", [R, 1], f32, kind="ExternalInput").ap()
    colfix1d = nc.dram_tensor("colfix1", [R, 1], f32, kind="ExternalInput").ap()
    out = nc.dram_tensor("out", [C_IMG, R, W], f32, kind="ExternalOutput").ap()

    with tile.TileContext(nc) as tc:
        with (
            tc.tile_pool(name="persist", bufs=1) as pp,
            tc.tile_pool(name="work", bufs=2) as wp,
            tc.tile_pool(name="io", bufs=2) as ip,
            tc.tile_pool(name="pstok", bufs=2, space="PSUM") as pst,
            tc.tile_pool(name="pslg", bufs=1, space="PSUM") as psl,
            tc.tile_pool(name="psatt", bufs=2, space="PSUM") as psa,
        ):
            # ---- persistent tiles ----
            imgsb = pp.tile([128, 2 * L * W], bf16, tag="imgsb")
            img3 = imgsb[:].rearrange("p (hh l w) -> p hh l w", hh=2, l=L)
            T3sum = [pp.tile([L, WP], bf16, tag=f"T3s_{k}", name=f"T3s_{k}")
                     for k in range(9)]
            data = [pp.tile([48, M], bf16, tag=f"d_{k}", name=f"d_{k}")
                    for k in range(9)]
            idxs_t = [pp.tile([48, Ms[s]], i16, tag=f"idxs{s}",
                              name=f"idxs{s}") for s in range(3)]
            bt_t = []
            for s in range(3):
                t = pp.tile([min(KS[s], 128), 9], bf16, tag=f"bt{s}",
                            name=f"bt{s}")
                nc.scalar.dma_start(out=t[:], in_=bts[s][:])
                bt_t.append(t)
            bt2b_t = pp.tile([KS[2] - 128, 9], bf16, tag="bt2b")
            nc.scalar.dma_start(out=bt2b_t[:], in_=bt2b[:])
            w3_t = pp.tile([C_IMG // 2, 2], bf16, tag="w3")
            nc.scalar.dma_start(out=w3_t[:], in_=w3d[:])
            ems = pp.tile([L, 384], bf16, tag="ems")
            nc.scalar.dma_start(out=ems[:], in_=emsd[:])
            rowmask_t = pp.tile([L, 1], f32, tag="rowmask")
            nc.scalar.dma_start(out=rowmask_t[:], in_=rowmaskd[:])
            rowfix_t = pp.tile([R, 1], f32, tag="rowfix")
            nc.scalar.dma_start(out=rowfix_t[:], in_=rowfixd[:])
            colfix0_t = pp.tile([R, 1], f32, tag="colfix0")
            nc.scalar.dma_start(out=colfix0_t[:], in_=colfix0d[:])
            colfix1_t = pp.tile([R, 1], f32, tag="colfix1")
            nc.scalar.dma_start(out=colfix1_t[:], in_=colfix1d[:])
            for s in range(3):
                nc.sync.dma_start(out=idxs_t[s][:], in_=idxs_d[s][:])

            ones_t = pp.tile([1, 128], bf16, tag="ones")
            nc.vector.memset(ones_t[:], 1.0)
            # local_scatter streams the full data tiles; zero the pad regions
            for k in range(9):
                nc.vector.memset(data[k][:], 0.0)
            gmap = pp.tile([L, WP], bf16, tag="gmap")
            nc.vector.memset(gmap[:], 0.0)
            att = pp.tile([R, W], bf16, tag="att")

            # ---- A: img slab cast-load on the SWDGE queue ----
            if no_cast:
                for hh in range(2):
                    for l in range(L):
                        imgf = ip.tile([128, W], f32, tag="out")
                        nc.sync.dma_start(
                            out=imgf[:],
                            in_=img[hh * 128:(hh + 1) * 128, l, :])
                        nc.scalar.copy(out=img3[:, hh, l, :], in_=imgf[:])
            else:
                row_chunks = [(0, 9), (9, 18), (18, 26), (26, L)]
                for r0, r1 in row_chunks:
                    for hh in range(2):
                        nc.gpsimd.dma_start(
                            out=img3[:, hh, r0:r1, :],
                            in_=img[hh * 128:(hh + 1) * 128, r0:r1, :
                                    ].rearrange("c l w -> c (l w)"))

            # ---- B+C: token matmuls -> data -> scatter -> T3 accum ----
            for s in range(3):
                NTs = -(-Ms[s] // TPT)
                stg = wp.tile([L, M * 9], bf16, tag="stg")
                for j in range(NTs):
                    tok = pst.tile([L, TPT * 9], f32, tag="tok")
                    t0 = TPT * j
                    t1 = min(t0 + TPT, Ms[s])
                    cols = (t1 - t0) * L
                    ka = min(KS[s], 128)
                    xt = wp.tile([128, TPT * L], bf16, tag=f"x{s}")
                    nc.sync.dma_start(out=xt[:ka, :cols],
                                      in_=xs[s][:, t0 * L:t1 * L])
                    xtb = None
                    if KS[s] > 128:
                        xtb = wp.tile([KS[s] - 128, TPT * L], bf16, tag="x2b")
                        nc.sync.dma_start(out=xtb[:, :cols],
                                          in_=x2b[:, t0 * L:t1 * L])
                    for t in range(t0, t1):
                        c0 = (t - t0) * L
                        po = (t - t0) * 9
                        dst = tok[:, po:po + 9]
                        if xtb is None:
                            nc.tensor.matmul(dst, xt[:ka, c0:c0 + L],
                                             bt_t[s][:], start=True, stop=True)
                        else:
                            nc.tensor.matmul(dst, xt[:, c0:c0 + L],
                                             bt_t[s][:], start=True, stop=False)
                            nc.tensor.matmul(dst, xtb[:, c0:c0 + L],
                                             bt2b_t[:], start=False, stop=True)
                    nc.vector.tensor_copy(
                        out=stg[:, j * TPT * 9:j * TPT * 9 + (t1 - t0) * 9],
                        in_=tok[:, :(t1 - t0) * 9])
                stg3 = stg[:].rearrange("p (t e) -> p t e", e=9)
                for k in range(9):
                    nc.vector.tensor_copy(out=data[k][0:L, :Ms[s]],
                                          in_=stg3[:, :Ms[s], k])
                for k in range(9):
                    tmp = wp.tile([48, WP], bf16, tag="t3tmp")
                    if no_lscat:
                        nc.vector.memset(tmp[:], 0.0)
                    else:
                        nc.gpsimd.local_scatter(
                            out_ap=tmp[:, :WP],
                            data_ap=data[k][:, :Ms[s]],
                            idxs_ap=idxs_t[s][:, :Ms[s]],
                            channels=48,
                            num_elems=WP,
                            num_idxs=Ms[s],
                        )
                    if s == 0:
                        nc.vector.tensor_copy(out=T3sum[k][:],
                                              in_=tmp[0:L, :])
                    else:
                        nc.vector.tensor_tensor(out=T3sum[k][:],
                                                in0=T3sum[k][:],
                                                in1=tmp[0:L, :],
                                                op=Alu.add)

            # ---- D: gated map ----
            for rc in range(L // 2):
                gstage = wp.tile([1, 2 * W], bf16, tag="gstage")
                for off, n in ESEGS:
                    gp = pst.tile([1, 512], f32, tag="g")
                    for hh in range(2):
                        nc.tensor.matmul(
                            gp[:, :n],
                            w3_t[:, hh:hh + 1],
                            img3[:, hh, 2 * rc:2 * rc + 2, :].rearrange(
                                "p l w -> p (l w)")[:, off:off + n],
                            start=(hh == 0), stop=(hh == 1))
                    nc.scalar.copy(out=gstage[:, off:off + n], in_=gp[:, :n])
                if safe_dma:
                    for r01 in range(2):
                        nc.scalar.dma_start(
                            out=gmap[2 * rc + r01:2 * rc + r01 + 1, 1:1 + W],
                            in_=gstage[:, r01 * W:(r01 + 1) * W])
                else:
                    nc.scalar.dma_start(
                        out=gmap[2 * rc:2 * rc + 2, 1:1 + W],
                        in_=gstage[:])
            # gmap = (gmap + b3) * rowmask on interior columns
            nc.vector.tensor_scalar(
                out=gmap[:, 1:1 + W], in0=gmap[:, 1:1 + W],
                scalar1=float(b3), scalar2=rowmask_t[:, 0:1],
                op0=Alu.add, op1=Alu.mult)

            # ---- E: logit assembly (em row-shift stationaries) ----
            lg = psl.tile([R, W], f32, tag="lg")
            nmm = [0, 0]
            total = 9 + 9
            for gi, (off, n) in enumerate(SEGS):
                for k in range(9):
                    dy, dx = divmod(k, 3)
                    nmm[gi] += 1
                    nc.tensor.matmul(
                        lg[:, off:off + n],
                        ems[:, dy * R:dy * R + R],
                        T3sum[k][:, dx + off:dx + off + n],
                        start=(nmm[gi] == 1), stop=False)
                    nmm[gi] += 1
                    nc.tensor.matmul(
                        lg[:, off:off + n],
                        ems[:, 96 + k * R:96 + k * R + R],
                        gmap[:, dx + off:dx + off + n],
                        start=False, stop=(nmm[gi] == total))
            nc.vector.tensor_tensor(out=lg[:, 0:1], in0=lg[:, 0:1],
                                    in1=colfix0_t[:, 0:1], op=Alu.add)
            nc.vector.tensor_tensor(out=lg[:, W - 1:W], in0=lg[:, W - 1:W],
                                    in1=colfix1_t[:, 0:1], op=Alu.add)
            # rowfix carries C_all + sp_b + row-edge constants
            for off, n in SEGS:
                nc.scalar.activation(att[:, off:off + n], lg[:, off:off + n],
                                     Act.Sigmoid, bias=rowfix_t[:, 0:1],
                                     scale=1.0)

            # ---- F: broadcast multiply + store ----
            for rc in range(R // 2):
                # flatten 2 att rows into one partition (sizes match; the
                # DMA walks both APs in row-major order)
                a1p = wp.tile([1, 2 * W], bf16, tag="a1p")
                if safe_dma:
                    for r01 in range(2):
                        nc.scalar.dma_start(
                            out=a1p[:, r01 * W:(r01 + 1) * W],
                            in_=att[2 * rc + r01:2 * rc + r01 + 1, :])
                else:
                    nc.scalar.dma_start(out=a1p[:],
                                        in_=att[2 * rc:2 * rc + 2, :])
                attb_sb = wp.tile([128, 2 * W], bf16, tag="attb_sb")
                for off, n in ESEGS:
                    ab = psa.tile([128, 512], f32, tag="attb")
                    nc.tensor.matmul(
                        ab[:, :n],
                        ones_t[:],
                        a1p[:, off:off + n],
                        start=True, stop=True)
                    nc.scalar.copy(out=attb_sb[:, off:off + n], in_=ab[:, :n])
                for hh in range(2):
                    for r01 in range(2):
                        ot = ip.tile([128, W], f32, tag="out")
                        nc.vector.tensor_tensor(
                            out=ot[:],
                            in0=img3[:, hh, 1 + 2 * rc + r01, :],
                            in1=attb_sb[:, r01 * W:(r01 + 1) * W],
                            op=Alu.mult)
                        eng = nc.sync if hh == 0 else nc.scalar
                        eng.dma_start(
                            out=out[hh * 128:(hh + 1) * 128,
                                    2 * rc + r01, :],
                            in_=ot[:])

    nc.compile()
    return nc


def _prepare(inputs):
    """Host-side fold + shard. Returns (Ms, M, b3, in_maps)."""
    import ml_dtypes
    bf = ml_dtypes.bfloat16

    fold = _fold_weights(inputs)
    cc, S = fold["cc"], fold["S"]

    grids = [np.asarray(inputs[f"img_grid_{i}"]) for i in range(3)]
    feats = [np.asarray(inputs[f"voxel_feat_{i}"]) for i in range(3)]
    coords = [np.asarray(inputs[f"voxel_coord_{i}"]) for i in range(3)]
    img_feat = np.asarray(inputs["img_feat"])

    # bucket tokens by (core, set, local row)
    sels = []          # sels[c][s] -> token ids
    hls = []           # matching local rows
    for c in range(N_CORES):
        lo = R * c - 1
        per, perh = [], []
        for s in range(3):
            rows = grids[s][:, 1]
            sel = np.nonzero((rows >= lo) & (rows < lo + L))[0]
            per.append(sel)
            perh.append(rows[sel] - lo)
        sels.append(per)
        hls.append(perh)

    Ms = []
    for s in range(3):
        mx = 0
        for c in range(N_CORES):
            if len(hls[c][s]):
                mx = max(mx, int(np.bincount(hls[c][s], minlength=L).max()))
        Ms.append(mx + (mx & 1))
    M = max(Ms)

    # em row-shift stationaries: [E_dy0|E_dy1|E_dy2 | S_0*E_dy(0) ...]
    ems = np.zeros((L, 384), np.float32)
    for k in range(9):
        dy = k // 3
        for r in range(R):
            ems[r + dy, dy * R + r] = 1.0
            ems[r + dy, 96 + k * R + r] = np.float32(S[k])

    in_maps = []
    for c in range(N_CORES):
        lo = R * c - 1
        m = {}
        slab = np.zeros((C_IMG, L, W), np.float32)
        g0, g1 = max(lo, 0), min(lo + L, H)
        slab[:, g0 - lo:g1 - lo, :] = img_feat[:, g0:g1, :]
        m["img"] = slab

        for s in range(3):
            sel = sels[c][s]
            hl = hls[c][s].astype(np.int64)
            wl = grids[s][sel, 0].astype(np.int64) + 1
            # slot position of each token within its row
            order = np.argsort(hl, kind="stable")
            hs = hl[order]
            slot = np.arange(len(hs)) - np.searchsorted(hs, hs, side="left")
            x = np.zeros((KS[s], Ms[s] * L), np.float32)
            idx = np.full((48, Ms[s]), -1, np.int16)
            if len(sel):
                xv = np.concatenate([feats[s][sel], coords[s][sel]],
                                    axis=1).T  # (K, n)
                cols = slot * L + hs
                x[:, cols] = xv[:, order]
                idx[hs, slot] = wl[order].astype(np.int16)
            m[f"idxs{s}"] = idx
            xb = x.astype(bf)
            if KS[s] > 128:
                m[f"x{s}"] = np.ascontiguousarray(xb[:128])
                m["x2b"] = np.ascontiguousarray(xb[128:])
            else:
                m[f"x{s}"] = xb
        for s in range(3):
            bt = fold["B"][s].T.astype(np.float32)  # (K, 9)
            if KS[s] > 128:
                m[f"bt{s}"] = np.ascontiguousarray(bt[:128]).astype(bf)
                m["bt2b"] = np.ascontiguousarray(bt[128:]).astype(bf)
            else:
                m[f"bt{s}"] = bt.astype(bf)
        m["w3"] = np.ascontiguousarray(
            fold["w3"].astype(np.float32).reshape(2, 128).T).astype(bf)
        m["ems"] = ems.astype(bf)
        rowmask = np.zeros((L, 1), np.float32)
        rowmask[g0 - lo:g1 - lo] = 1.0
        m["rowmask"] = rowmask
        # rowfix: C_all + sp_b + row-edge constants (used as sigmoid bias)
        rowfix = np.full((R, 1), fold["C_all"] + fold["spb"], np.float64)
        colfix0 = np.full((R, 1), -(cc[0] + cc[3] + cc[6]))
        colfix1 = np.full((R, 1), -(cc[2] + cc[5] + cc[8]))
        for hloc in range(R):
            g = R * c + hloc
            if g == 0:
                rowfix[hloc] += -(cc[0] + cc[1] + cc[2])
                colfix0[hloc] += cc[0]
                colfix1[hloc] += cc[2]
            if g == H - 1:
                rowfix[hloc] += -(cc[6] + cc[7] + cc[8])
                colfix0[hloc] += cc[6]
                colfix1[hloc] += cc[8]
        m["rowfix"] = rowfix.astype(np.float32)
        m["colfix0"] = colfix0.astype(np.float32)
        m["colfix1"] = colfix1.astype(np.float32)
        in_maps.append(m)
    return Ms, M, fold["b3"], in_maps


def kernel(**inputs):
    global LAST_RESULT
    from concourse.bass_utils import run_bass_kernel_spmd

    Ms, M, b3, in_maps = _prepare(inputs)
    nc = _build_program(Ms, M, b3)
    res = run_bass_kernel_spmd(nc, in_maps, core_ids=list(range(N_CORES)))
    LAST_RESULT = res
    out = np.concatenate(
        [res.results[c]["out"] for c in range(N_CORES)], axis=1)
    return np.ascontiguousarray(out.astype(np.float32))


# revision 38
# speedup vs baseline: 3.4577x; 1.0838x over previous
"""Trainium2 Bass kernel for the Basicgate multivoxel attention module.

The voxel-features -> attention-logit chain is linear and collapses to

  logit(r,w) = sum_{s,k} T_s[k, r+dy-1, w+dx-1]          (point taps)
             + sum_k S[k] * gated(r+dy-1, w+dx-1)        (gated 3x3)
             + edge constants;     out = img * sigmoid(logit + sp_b)

with per point p of set s at cell (h,w): T_s[:, h, w] = B_s @ x_p,
B_s = V@W2@W0 / V@W2@W1 / V@W2, x_p = concat(feat, coord), V the 3x3 taps,
gated = w3.img + b3, S[k] = sum_c V[k,c].

Sharding: H split across 8 cores (32 owned rows + 1 halo row per side ->
34 local rows).  Points bucketed on host by (core, set, local row).  No
collectives; host gathers the output.

Device pipeline per core (all dense data bf16, accumulation f32 in PSUM):
  A. img slab (256,34,704) f32 is DMA'd ONCE via the gpsimd (SWDGE) queue
     with an f32->bf16 cast into a persistent SBUF slab (95KB/partition).
  B. token matmuls: stationary is B_s^T [K,9] (loaded once per set),
     moving is x [K,512] with host-ordered row-major tokens -> PSUM
     [9 taps, tokens] in ~70 fat matmuls (per-slot stationaries cost
     ~390ns of pipeline overhead each).  DVE/ACT copy PSUM to a bf16
     staging row-stream; one 34-descriptor SBUF->SBUF DMA per (set,tap)
     reshapes it into data_k[row, slot] (DMAs may cross partitions;
     engines may not).
  C. gpsimd.local_scatter x27 (tap x set): a [48,706] temp tile gets
     data_k placed at host-provided column indices; DVE accumulates the
     three sets into T3sum_k.  The Q7 scatter kernel vectorizes in GPSIMD
     local RAM (~15 cycles per 32 tokens, one DSP per 16 partitions),
     zero-fills the dst, and skips negative (padding) indices -- it
     replaces the baseline's dma_scatter_add (7ns/token serial desc-gen)
     + 18MB DRAM zeroing + 36B-descriptor readback, which dominated the
     baseline runtime.
  D. gated map: w3 . img streamed from the bf16 slab on PE.
  E. logit: 9 T taps + 9 gated taps accumulate into PSUM as matmuls with
     row-shift selection stationaries (em_dy, S_k-scaled for gated).
     Edge-constant column/row fixes, then sigmoid (ACT, bias=rowfix) ->
     att bf16.
  F. att broadcast to 128 partitions (ones-stationary matmul), DVE
     multiply with the bf16 img slab, f32 out stores split across the two
     HWDGE queues.
"""

import numpy as np

# ---- problem constants (hardcoded per contract) ----
C_IMG = 256
H, W = 256, 704
CH = [32, 64, 128]
COUT = 131
N_CORES = 8
R = 32            # owned rows per core
L = 34            # local rows incl 1-row halo each side
WP = W + 2        # padded width (token cols live in [1, 705))
KS = [CH[0] + 3, CH[1] + 3, COUT]   # 35, 67, 131
P102 = 3 * L      # (set, row) partitions
P112 = 112        # padded to a multiple of 16 for local_scatter channels
TPT = 28          # token slots per PSUM tile (28*9 = 252 f32 cols)

LAST_RESULT = None  # stash of BassKernelResults for the test harness


def _fold_weights(inputs):
    f8 = np.float64
    W0 = inputs["rd0_w"][:, :, 0, 0].astype(f8)   # (131, 35)
    W1 = inputs["rd1_w"][:, :, 0, 0].astype(f8)   # (131, 67)
    W2 = inputs["rd2_w"][:, :, 0, 0].astype(f8)   # (131, 131)
    w3 = inputs["rd3_w"][0, :, 0, 0].astype(f8)   # (256,)
    b0 = inputs["rd0_b"].astype(f8)
    b1 = inputs["rd1_b"].astype(f8)
    b2 = inputs["rd2_b"].astype(f8)
    b3 = float(inputs["rd3_b"][0])
    spb = float(inputs["sp_b"][0])
    # V[k=dy*3+dx, c] = sp_w[0, c, dy, dx]
    V = inputs["sp_w"][0].astype(f8).transpose(1, 2, 0).reshape(9, COUT)
    B = [V @ (W2 @ W0), V @ (W2 @ W1), V @ W2]
    cc = V @ (W2 @ (b0 + b1) + b2)   # (9,)
    S = V.sum(axis=1)                # (9,)
    return dict(B=B, cc=cc, S=S, C_all=float(cc.sum()),
                w3=w3, b3=b3, spb=spb)


def _build_program(Ms, M, b3):
    """Ms = per-set token-slot capacity per row (identical across cores),
    M = max(Ms[0], Ms[1]) and M2 = Ms[2], both even (local_scatter
    num_idxs)."""
    import concourse.bacc as bacc
    import concourse.mybir as mybir
    import concourse.tile as tile

    f32 = mybir.dt.float32
    bf16 = mybir.dt.bfloat16
    i16 = mybir.dt.int16
    Alu = mybir.AluOpType
    Act = mybir.ActivationFunctionType

    import os
    no_cast = os.environ.get("BG_NO_CAST", "0") == "1"
    no_lscat = os.environ.get("BG_NO_LSCAT", "0") == "1"
    safe_dma = os.environ.get("BG_SAFE_DMA", "0") == "1"

    SEGS = ((0, 512), (512, 192))
    ESEGS = ((0, 512), (512, 512), (1024, 384))

    nc = bacc.Bacc("TRN2", target_bir_lowering=False, debug=False,
                   num_devices=N_CORES)

    img = nc.dram_tensor("img", [C_IMG, L, W], f32, kind="ExternalInput").ap()
    xs = [nc.dram_tensor(f"x{i}", [min(KS[i], 128), Ms[i] * L], bf16,
                         kind="ExternalInput").ap() for i in range(3)]
    x2b = nc.dram_tensor("x2b", [KS[2] - 128, Ms[2] * L], bf16,
                         kind="ExternalInput").ap()
    idxs_d = [nc.dram_tensor(f"idxs{s}", [48, Ms[s]], i16,
                             kind="ExternalInput").ap() for s in range(3)]
    bts = [nc.dram_tensor(f"bt{i}", [min(KS[i], 128), 9], bf16,
                          kind="ExternalInput").ap() for i in range(3)]
    bt2b = nc.dram_tensor("bt2b", [KS[2] - 128, 9], bf16,
                          kind="ExternalInput").ap()
    w3d = nc.dram_tensor("w3", [C_IMG // 2, 2], bf16, kind="ExternalInput").ap()
    emsd = nc.dram_tensor("ems", [L, 384], bf16, kind="ExternalInput").ap()
    rowmaskd = nc.dram_tensor("rowmask", [L, 1], f32, kind="ExternalInput").ap()
    rowfixd = nc.dram_tensor("rowfix", [R, 1], f32, kind="ExternalInput").ap()
    colfix0d = nc.dram_tensor("colfix0", [R, 1], f32, kind="ExternalInput").ap()
    colfix1d = nc.dram_tensor("colfix1", [R, 1], f32, kind="ExternalInput").ap()
    out = nc.dram_tensor("out", [C_IMG, R, W], f32, kind="ExternalOutput").ap()

    with tile.TileContext(nc) as tc:
        with (
            tc.tile_pool(name="persist", bufs=1) as pp,
            tc.tile_pool(name="work", bufs=2) as wp,
            tc.tile_pool(name="io", bufs=2) as ip,
            tc.tile_pool(name="pstok", bufs=2, space="PSUM") as pst,
            tc.tile_pool(name="pslg", bufs=1, space="PSUM") as psl,
            tc.tile_pool(name="psatt", bufs=2, space="PSUM") as psa,
        ):
            # ---- persistent tiles ----
            imgsb = pp.tile([128, 2 * L * W], bf16, tag="imgsb")
            img3 = imgsb[:].rearrange("p (hh l w) -> p hh l w", hh=2, l=L)
            T3sum = [pp.tile([L, WP], bf16, tag=f"T3s_{k}", name=f"T3s_{k}")
                     for k in range(9)]
            data = [pp.tile([48, M], bf16, tag=f"d_{k}", name=f"d_{k}")
                    for k in range(9)]
            idxs_t = [pp.tile([48, Ms[s]], i16, tag=f"idxs{s}",
                              name=f"idxs{s}") for s in range(3)]
            bt_t = []
            for s in range(3):
                t = pp.tile([min(KS[s], 128), 9], bf16, tag=f"bt{s}",
                            name=f"bt{s}")
                nc.scalar.dma_start(out=t[:], in_=bts[s][:])
                bt_t.append(t)
            bt2b_t = pp.tile([KS[2] - 128, 9], bf16, tag="bt2b")
            nc.scalar.dma_start(out=bt2b_t[:], in_=bt2b[:])
            w3_t = pp.tile([C_IMG // 2, 2], bf16, tag="w3")
            nc.scalar.dma_start(out=w3_t[:], in_=w3d[:])
            ems = pp.tile([L, 384], bf16, tag="ems")
            nc.scalar.dma_start(out=ems[:], in_=emsd[:])
            rowmask_t = pp.tile([L, 1], f32, tag="rowmask")
            nc.scalar.dma_start(out=rowmask_t[:], in_=rowmaskd[:])
            rowfix_t = pp.tile([R, 1], f32, tag="rowfix")
            nc.scalar.dma_start(out=rowfix_t[:], in_=rowfixd[:])
            colfix0_t = pp.tile([R, 1], f32, tag="colfix0")
            nc.scalar.dma_start(out=colfix0_t[:], in_=colfix0d[:])
            colfix1_t = pp.tile([R, 1], f32, tag="colfix1")
            nc.scalar.dma_start(out=colfix1_t[:], in_=colfix1d[:])
            for s in range(3):
                nc.sync.dma_start(out=idxs_t[s][:], in_=idxs_d[s][:])

            ones_t = pp.tile([1, 128], bf16, tag="ones")
            nc.vector.memset(ones_t[:], 1.0)
            # local_scatter streams the full data tiles; zero the pad regions
            for k in range(9):
                nc.vector.memset(data[k][:], 0.0)
            gmap = pp.tile([L, WP], bf16, tag="gmap")
            nc.vector.memset(gmap[:], 0.0)
            att = pp.tile([R, W], bf16, tag="att")

            # ---- A: img slab cast-load on the SWDGE queue ----
            if no_cast:
                for hh in range(2):
                    for l in range(L):
                        imgf = ip.tile([128, W], f32, tag="out")
                        nc.sync.dma_start(
                            out=imgf[:],
                            in_=img[hh * 128:(hh + 1) * 128, l, :])
                        nc.scalar.copy(out=img3[:, hh, l, :], in_=imgf[:])
            else:
                row_chunks = [(0, 9), (9, 18), (18, 26), (26, L)]
                for r0, r1 in row_chunks:
                    for hh in range(2):
                        nc.gpsimd.dma_start(
                            out=img3[:, hh, r0:r1, :],
                            in_=img[hh * 128:(hh + 1) * 128, r0:r1, :
                                    ].rearrange("c l w -> c (l w)"))

            # ---- B+C: token matmuls -> staging -> data -> scatter ----
            staging = pp.tile([9, L * M], bf16, tag="stage9")
            XC = 512
            for s in range(3):
                ntok = L * Ms[s]
                ka = min(KS[s], 128)
                for j in range(-(-ntok // XC)):
                    c0 = XC * j
                    cols = min(XC, ntok - c0)
                    tok = pst.tile([9, XC], f32, tag="tok")
                    xt = wp.tile([128, XC], bf16, tag=f"x{s}")
                    nc.sync.dma_start(out=xt[:ka, :cols],
                                      in_=xs[s][:, c0:c0 + cols])
                    if KS[s] > 128:
                        xtb = wp.tile([KS[s] - 128, XC], bf16, tag="x2b")
                        nc.sync.dma_start(out=xtb[:, :cols],
                                          in_=x2b[:, c0:c0 + cols])
                        nc.tensor.matmul(tok[:, :cols], bt_t[s][:],
                                         xt[:, :cols], start=True, stop=False)
                        nc.tensor.matmul(tok[:, :cols], bt2b_t[:],
                                         xtb[:, :cols], start=False, stop=True)
                    else:
                        nc.tensor.matmul(tok[:, :cols], bt_t[s][:ka, :],
                                         xt[:ka, :cols], start=True, stop=True)
                    eng = nc.vector if j % 2 == 0 else nc.scalar
                    if j % 2 == 0:
                        nc.vector.tensor_copy(out=staging[:, c0:c0 + cols],
                                              in_=tok[:, :cols])
                    else:
                        nc.scalar.copy(out=staging[:, c0:c0 + cols],
                                       in_=tok[:, :cols])
                for k in range(9):
                    # [1, 34*Ms] row-stream -> [34, Ms]: same element order
                    eng = nc.sync if k % 2 == 0 else nc.scalar
                    eng.dma_start(out=data[k][0:L, :Ms[s]],
                                  in_=staging[k:k + 1, :ntok])
                for k in range(9):
                    tmp = wp.tile([48, WP], bf16, tag="t3tmp")
                    if no_lscat:
                        nc.vector.memset(tmp[:], 0.0)
                    else:
                        nc.gpsimd.local_scatter(
                            out_ap=tmp[:, :WP],
                            data_ap=data[k][:, :Ms[s]],
                            idxs_ap=idxs_t[s][:, :Ms[s]],
                            channels=48,
                            num_elems=WP,
                            num_idxs=Ms[s],
                        )
                    if s == 0:
                        nc.vector.tensor_copy(out=T3sum[k][:],
                                              in_=tmp[0:L, :])
                    else:
                        nc.vector.tensor_tensor(out=T3sum[k][:],
                                                in0=T3sum[k][:],
                                                in1=tmp[0:L, :],
                                                op=Alu.add)

            # ---- D: gated map ----
            for rc in range(L // 2):
                gstage = wp.tile([1, 2 * W], bf16, tag="gstage")
                for off, n in ESEGS:
                    gp = pst.tile([1, 512], f32, tag="g")
                    for hh in range(2):
                        nc.tensor.matmul(
                            gp[:, :n],
                            w3_t[:, hh:hh + 1],
                            img3[:, hh, 2 * rc:2 * rc + 2, :].rearrange(
                                "p l w -> p (l w)")[:, off:off + n],
                            start=(hh == 0), stop=(hh == 1))
                    nc.scalar.copy(out=gstage[:, off:off + n], in_=gp[:, :n])
                if safe_dma:
                    for r01 in range(2):
                        nc.scalar.dma_start(
                            out=gmap[2 * rc + r01:2 * rc + r01 + 1, 1:1 + W],
                            in_=gstage[:, r01 * W:(r01 + 1) * W])
                else:
                    nc.scalar.dma_start(
                        out=gmap[2 * rc:2 * rc + 2, 1:1 + W],
                        in_=gstage[:])
            # gmap = (gmap + b3) * rowmask on interior columns
            nc.vector.tensor_scalar(
                out=gmap[:, 1:1 + W], in0=gmap[:, 1:1 + W],
                scalar1=float(b3), scalar2=rowmask_t[:, 0:1],
                op0=Alu.add, op1=Alu.mult)

            # ---- E: logit assembly (em row-shift stationaries) ----
            lg = psl.tile([R, W], f32, tag="lg")
            nmm = [0, 0]
            total = 9 + 9
            for gi, (off, n) in enumerate(SEGS):
                for k in range(9):
                    dy, dx = divmod(k, 3)
                    nmm[gi] += 1
                    nc.tensor.matmul(
                        lg[:, off:off + n],
                        ems[:, dy * R:dy * R + R],
                        T3sum[k][:, dx + off:dx + off + n],
                        start=(nmm[gi] == 1), stop=False)
                    nmm[gi] += 1
                    nc.tensor.matmul(
                        lg[:, off:off + n],
                        ems[:, 96 + k * R:96 + k * R + R],
                        gmap[:, dx + off:dx + off + n],
                        start=False, stop=(nmm[gi] == total))
            nc.vector.tensor_tensor(out=lg[:, 0:1], in0=lg[:, 0:1],
                                    in1=colfix0_t[:, 0:1], op=Alu.add)
            nc.vector.tensor_tensor(out=lg[:, W - 1:W], in0=lg[:, W - 1:W],
                                    in1=colfix1_t[:, 0:1], op=Alu.add)
            # rowfix carries C_all + sp_b + row-edge constants
            for off, n in SEGS:
                nc.scalar.activation(att[:, off:off + n], lg[:, off:off + n],
                                     Act.Sigmoid, bias=rowfix_t[:, 0:1],
                                     scale=1.0)

            # ---- F: broadcast multiply + store ----
            for rc in range(R // 2):
                # flatten 2 att rows into one partition (sizes match; the
                # DMA walks both APs in row-major order)
                a1p = wp.tile([1, 2 * W], bf16, tag="a1p")
                if safe_dma:
                    for r01 in range(2):
                        nc.scalar.dma_start(
                            out=a1p[:, r01 * W:(r01 + 1) * W],
                            in_=att[2 * rc + r01:2 * rc + r01 + 1, :])
                else:
                    nc.scalar.dma_start(out=a1p[:],
                                        in_=att[2 * rc:2 * rc + 2, :])
                attb_sb = wp.tile([128, 2 * W], bf16, tag="attb_sb")
                for off, n in ESEGS:
                    ab = psa.tile([128, 512], f32, tag="attb")
                    nc.tensor.matmul(
                        ab[:, :n],
                        ones_t[:],
                        a1p[:, off:off + n],
                        start=True, stop=True)
                    nc.scalar.copy(out=attb_sb[:, off:off + n], in_=ab[:, :n])
                for hh in range(2):
                    for r01 in range(2):
                        ot = ip.tile([128, W], f32, tag="out")
                        nc.vector.tensor_tensor(
                            out=ot[:],
                            in0=img3[:, hh, 1 + 2 * rc + r01, :],
                            in1=attb_sb[:, r01 * W:(r01 + 1) * W],
                            op=Alu.mult)
                        eng = nc.sync if hh == 0 else nc.scalar
                        eng.dma_start(
                            out=out[hh * 128:(hh + 1) * 128,
                                    2 * rc + r01, :],
                            in_=ot[:])

    nc.compile()
    return nc


def _prepare(inputs):
    """Host-side fold + shard. Returns (Ms, M, b3, in_maps)."""
    import ml_dtypes
    bf = ml_dtypes.bfloat16

    fold = _fold_weights(inputs)
    cc, S = fold["cc"], fold["S"]

    grids = [np.asarray(inputs[f"img_grid_{i}"]) for i in range(3)]
    feats = [np.asarray(inputs[f"voxel_feat_{i}"]) for i in range(3)]
    coords = [np.asarray(inputs[f"voxel_coord_{i}"]) for i in range(3)]
    img_feat = np.asarray(inputs["img_feat"])

    # bucket tokens by (core, set, local row)
    sels = []          # sels[c][s] -> token ids
    hls = []           # matching local rows
    for c in range(N_CORES):
        lo = R * c - 1
        per, perh = [], []
        for s in range(3):
            rows = grids[s][:, 1]
            sel = np.nonzero((rows >= lo) & (rows < lo + L))[0]
            per.append(sel)
            perh.append(rows[sel] - lo)
        sels.append(per)
        hls.append(perh)

    Ms = []
    for s in range(3):
        mx = 0
        for c in range(N_CORES):
            if len(hls[c][s]):
                mx = max(mx, int(np.bincount(hls[c][s], minlength=L).max()))
        Ms.append(mx + (mx & 1))
    M = max(Ms)

    # em row-shift stationaries: [E_dy0|E_dy1|E_dy2 | S_0*E_dy(0) ...]
    ems = np.zeros((L, 384), np.float32)
    for k in range(9):
        dy = k // 3
        for r in range(R):
            ems[r + dy, dy * R + r] = 1.0
            ems[r + dy, 96 + k * R + r] = np.float32(S[k])

    in_maps = []
    for c in range(N_CORES):
        lo = R * c - 1
        m = {}
        slab = np.zeros((C_IMG, L, W), np.float32)
        g0, g1 = max(lo, 0), min(lo + L, H)
        slab[:, g0 - lo:g1 - lo, :] = img_feat[:, g0:g1, :]
        m["img"] = slab

        for s in range(3):
            sel = sels[c][s]
            hl = hls[c][s].astype(np.int64)
            wl = grids[s][sel, 0].astype(np.int64) + 1
            # slot position of each token within its row
            order = np.argsort(hl, kind="stable")
            hs = hl[order]
            slot = np.arange(len(hs)) - np.searchsorted(hs, hs, side="left")
            x = np.zeros((KS[s], Ms[s] * L), np.float32)
            idx = np.full((48, Ms[s]), -1, np.int16)
            if len(sel):
                xv = np.concatenate([feats[s][sel], coords[s][sel]],
                                    axis=1).T  # (K, n)
                cols = hs * Ms[s] + slot
                x[:, cols] = xv[:, order]
                idx[hs, slot] = wl[order].astype(np.int16)
            m[f"idxs{s}"] = idx
            xb = x.astype(bf)
            if KS[s] > 128:
                m[f"x{s}"] = np.ascontiguousarray(xb[:128])
                m["x2b"] = np.ascontiguousarray(xb[128:])
            else:
                m[f"x{s}"] = xb
        for s in range(3):
            bt = fold["B"][s].T.astype(np.float32)  # (K, 9)
            if KS[s] > 128:
                m[f"bt{s}"] = np.ascontiguousarray(bt[:128]).astype(bf)
                m["bt2b"] = np.ascontiguousarray(bt[128:]).astype(bf)
            else:
                m[f"bt{s}"] = bt.astype(bf)
        m["w3"] = np.ascontiguousarray(
            fold["w3"].astype(np.float32).reshape(2, 128).T).astype(bf)
        m["ems"] = ems.astype(bf)
        rowmask = np.zeros((L, 1), np.float32)
        rowmask[g0 - lo:g1 - lo] = 1.0
        m["rowmask"] = rowmask
        # rowfix: C_all + sp_b + row-edge constants (used as sigmoid bias)
        rowfix = np.full((R, 1), fold["C_all"] + fold["spb"], np.float64)
        colfix0 = np.full((R, 1), -(cc[0] + cc[3] + cc[6]))
        colfix1 = np.full((R, 1), -(cc[2] + cc[5] + cc[8]))
        for hloc in range(R):
            g = R * c + hloc
            if g == 0:
                rowfix[hloc] += -(cc[0] + cc[1] + cc[2])
                colfix0[hloc] += cc[0]
                colfix1[hloc] += cc[2]
            if g == H - 1:
                rowfix[hloc] += -(cc[6] + cc[7] + cc[8])
                colfix0[hloc] += cc[6]
                colfix1[hloc] += cc[8]
        m["rowfix"] = rowfix.astype(np.float32)
        m["colfix0"] = colfix0.astype(np.float32)
        m["colfix1"] = colfix1.astype(np.float32)
        in_maps.append(m)
    return Ms, M, fold["b3"], in_maps


def kernel(**inputs):
    global LAST_RESULT
    from concourse.bass_utils import run_bass_kernel_spmd

    Ms, M, b3, in_maps = _prepare(inputs)
    nc = _build_program(Ms, M, b3)
    res = run_bass_kernel_spmd(nc, in_maps, core_ids=list(range(N_CORES)))
    LAST_RESULT = res
    out = np.concatenate(
        [res.results[c]["out"] for c in range(N_CORES)], axis=1)
    return np.ascontiguousarray(out.astype(np.float32))
